# revision 1
# baseline (speedup 1.0000x reference)
"""Trainium2 Bass kernel for nn_MultiHeadAttention_56118042690041.

8-core sharding: batch x heads tensor-parallel.
  core c (0..7): batch b = c//4, heads 4*(c%4) .. 4*(c%4)+4 (as 2 packed pairs).
Per core:
  - QKV projections for its 4 heads (head-pairs packed to M=128), contraction
    over D in PSUM, fp32r matmuls.
  - Attention per head (note reference's faithful "bug": scores = v2 @ k2^T,
    weighted sum of q2): scoresT[t,s] tiles on PE (K=dk=64), exp on ACT
    (no max-subtraction needed: scores are tiny by construction), AV matmul
    with a ones-column augmented q2 giving the softmax denominator for free,
    normalization on DVE.
  - Head outputs (headoutT layout [dk, s]) AllGather'd across the 4 cores of
    the same batch group -> full [H*DK, S] per core.
  - Output projection: each core computes a disjoint 256-wide d-slice of
    out = headout @ Wo^T + bo (column-sharded Wo -> SPMD-uniform program).
Host: slices weights per core, transposes x, concatenates disjoint outputs.
"""

import contextlib
import ctypes
import os
import sys
import types

import numpy as np

if "/opt/trn_rl_repo" not in sys.path:
    sys.path.insert(0, "/opt/trn_rl_repo")

# ---------------------------------------------------------------- shims ----


def _install_antenv_shim():
    """Provide antenv.axon_hooks (NTFF profile hook) if the image lacks it."""
    try:
        import antenv.axon_hooks  # noqa: F401

        return
    except ImportError:
        pass

    def _hook_factory():
        so_path = "/opt/axon/libaxon_pjrt.so"
        try:
            lib = ctypes.CDLL(so_path)
        except OSError:
            return None
        if not hasattr(lib, "axon_start_nrt_profile"):
            return None
        lib.axon_start_nrt_profile.argtypes = [
            ctypes.POINTER(ctypes.c_int64),
            ctypes.c_size_t,
        ]
        lib.axon_start_nrt_profile.restype = ctypes.c_int64
        lib.axon_stop_nrt_profile.argtypes = [ctypes.c_char_p]
        lib.axon_stop_nrt_profile.restype = ctypes.c_int64

        @contextlib.contextmanager
        def _hook(output_dir, device_ids):
            import jax

            jax.devices()
            if device_ids:
                ids = (ctypes.c_int64 * len(device_ids))(*device_ids)
                rc = lib.axon_start_nrt_profile(ids, len(device_ids))
            else:
                rc = lib.axon_start_nrt_profile(None, 0)
            if rc != 0:
                raise RuntimeError(f"axon_start_nrt_profile rc={rc}")
            try:
                yield
            finally:
                n = lib.axon_stop_nrt_profile(str(output_dir).encode())
                print(f"ntff profile: {n} file(s) -> {output_dir}", file=sys.stderr)

        return _hook

    hook = _hook_factory()
    mod = types.ModuleType("antenv.axon_hooks")
    mod.get_axon_ntff_profile_hook = lambda: hook
    mod.set_axon_ntff_profile_hook = lambda h: None
    sys.modules["antenv.axon_hooks"] = mod


def _install_tile_drain_patch():
    """This walrus build rejects >1 sync wait on the Tile tail Drain; split the
    waits across chained single-wait drains."""
    import concourse.tile as tile

    if getattr(tile.TileContext, "_drain_patch_installed", False):
        return

    def _drain_and_barrier(self, tick_clock, wait_clock):
        nc = self.nc
        drain_inst = nc.sync.drain()
        wait_clock.add_sem_waits(
            drain_inst.ins, tile.ScopedClock({None: tick_clock.global_clock})
        )
        si = drain_inst.ins.sync_info
        waits = list(si.on_wait) if si is not None and si.on_wait else []
        if len(waits) > 1:
            si.on_wait = waits[:1]
            assert self.sems is not None
            by_num = {h.num: h for h in self.sems.allocated().values()}
            for w in waits[1:]:
                d2 = nc.sync.drain()
                h = by_num.get(w.id)
                assert h is not None, f"no sem handle for wait {w.ant_name}"
                d2.wait_op(h, w.wait_value, "sem-ge", check=False)
        nc.all_engine_barrier()
        assert self.sems is not None
        popped = nc._tile_sem_poison_stack.pop()
        assert popped is self._sem_poison
        nc.clear_and_free_semaphores(list(self.sems.allocated().values()))
        nc.all_engine_barrier()

    tile.TileContext._drain_and_barrier = _drain_and_barrier
    tile.TileContext._drain_patch_installed = True


_install_antenv_shim()


def _split_multi_waits(nc, max_waits=1):
    """This walrus build rejects instructions carrying more than ~1 sync wait.
    Move excess waits onto same-engine NOPs inserted immediately before the
    instruction (sequencer waits execute in stream order, so this is
    semantics-preserving)."""
    import bass_rust
    import concourse.mybir as mybir

    n = 0
    for bb in nc.m.functions[0].blocks:
        insts = bb.instructions
        out = []
        for inst in insts:
            si = inst.sync_info
            waits = list(si.on_wait) if si is not None and si.on_wait else []
            if len(waits) > max_waits:
                keep = waits[-max_waits:]
                for w in waits[:-max_waits]:
                    nop = mybir.InstNoOp(name=f"waitnop_{n}", ins=[], outs=[])
                    n += 1
                    nop.engine = inst.engine
                    nop.sync_info = bass_rust.SyncInfo(on_wait=[w], on_update=[])
                    out.append(nop)
                si.on_wait = keep
            out.append(inst)
        if len(out) != len(insts):
            insts[:] = out
    return n


# ------------------------------------------------------------- program -----

N_CORES = 8
GROUP = 4  # cores per batch group
USE_FP32R = True
ATTN_BF16 = True

last_results = None  # BassKernelResults of the most recent run (for test.py)


def build_program(S=2048, DM=1024, H=16, DK=64, use_fp32r=USE_FP32R, attn_bf16=ATTN_BF16, split_waits=True):
    """Emit the SPMD Bass/Tile program. Returns nc."""
    import concourse.bass as bass
    import concourse.mybir as mybir
    import concourse.tile as tile

    _install_tile_drain_patch()

    f32 = mybir.dt.float32
    f32r = mybir.dt.float32r
    NPAIR = 2  # head pairs per core (4 heads)
    KT = DM // 128  # contraction tiles for projections
    TT = S // 128  # t tiles (scores row blocks / AV contraction tiles)
    SQ = min(1024, S)  # scores/exp free width
    NSH = S // SQ
    SB = min(512, SQ)  # AV / normalize block
    NSB2 = SQ // SB
    MMN = min(512, S)  # matmul moving max (fp32)
    HDK = H * DK  # concat dim (1024)
    KO = HDK // 128  # outproj contraction tiles
    DSL = HDK // GROUP  # out d-slice per core (256)
    OSB = S // 128  # outproj s blocks

    nc = bass.Bass(
        trn_type="TRN2", target_bir_lowering=False, debug=False, num_devices=N_CORES
    )

    def din(name, shape):
        return nc.dram_tensor(name, shape, f32, kind="ExternalInput").ap()

    xT = {p: din(f"x{p}T", [DM, S]) for p in ("q", "k", "v")}  # x[b].T per kind
    W = {p: din(f"w{p}", [NPAIR, DM, 128]) for p in ("q", "k", "v")}  # pair-packed W.T
    bq = din("bq", [NPAIR, 128, 1])
    bk8 = din("bk8", [NPAIR, 128, 1])  # bk / sqrt(dk)
    bv = din("bv", [NPAIR, 128, 1])
    woT = din("woT", [HDK, DSL])  # Wo.T columns for this core's d-slice
    boT = din("boT", [128, 2])  # bo d-slice as [128, 2] (col = 128-wide d block)
    ident = din("ident", [128, 64])  # eye(64) stacked twice (both partition halves)
    out_ap = nc.dram_tensor("out", [DSL, S], f32, kind="ExternalOutput").ap()

    fr = f32r if use_fp32r else f32  # dtype for matmul operand tiles
    fa = mybir.dt.bfloat16 if attn_bf16 else fr  # attention matmul operand dtype

    with tile.TileContext(nc) as tc:
        with contextlib.ExitStack() as ctx:
            sb = ctx.enter_context(tc.tile_pool(name="sb", bufs=2))
            big = ctx.enter_context(tc.tile_pool(name="big", bufs=8))
            ps = ctx.enter_context(tc.tile_pool(name="ps", bufs=2, space="PSUM"))
            dram = ctx.enter_context(tc.tile_pool(name="dram", bufs=1, space="DRAM"))

            # --- constants / small tiles ---
            ident_sb = sb.tile([128, 64], f32, tag="ident", bufs=1)
            nc.sync.dma_start(ident_sb[:], ident[:])
            ones64 = sb.tile([1, 64], f32, tag="ones", bufs=1)
            nc.gpsimd.memset(ones64[:], 1.0)
            ones128 = sb.tile([128, 1], f32, tag="ones1", bufs=1)
            nc.gpsimd.memset(ones128[:], 1.0)
            bq_sb = sb.tile([128, NPAIR], f32, tag="bq", bufs=1)
            bk_sb = sb.tile([128, NPAIR], f32, tag="bk", bufs=1)
            bv_sb = sb.tile([128, NPAIR], f32, tag="bv", bufs=1)
            for p in range(NPAIR):
                nc.sync.dma_start(bq_sb[:, p : p + 1], bq[p])
                nc.sync.dma_start(bk_sb[:, p : p + 1], bk8[p])
                nc.sync.dma_start(bv_sb[:, p : p + 1], bv[p])
            boT_sb = sb.tile([128, 2], f32, tag="bo", bufs=1)
            nc.sync.dma_start(boT_sb[:], boT[:])
            woT_sb = sb.tile([128, KO * DSL], fr, tag="wo", bufs=1)
            for k in range(KO):
                nc.sync.dma_start(
                    woT_sb[:, k * DSL : (k + 1) * DSL],
                    woT[k * 128 : (k + 1) * 128, :].bitcast(fr),
                )

            # --- phase P: projections -> q2T/k2T/v2T pair tiles [128, S] ---
            proj_out = {}
            for kind, bias_kind in (("v", "v"), ("k", "k"), ("q", "q")):
                w_sb = [
                    sb.tile([128, KT * 128], fr, tag="w", bufs=4, name=f"w_{kind}{p}")
                    for p in range(NPAIR)
                ]
                for p in range(NPAIR):
                    for k in range(KT):
                        nc.sync.dma_start(
                            w_sb[p][:, k * 128 : (k + 1) * 128],
                            W[kind][p, k * 128 : (k + 1) * 128, :].bitcast(fr),
                        )
                odt = f32 if kind == "q" else fa
                otag = "big2048" if kind == "q" else "big2048h"
                outs = [
                    big.tile([128, S], odt, tag=otag, bufs=4, name=f"{kind}2T_{p}")
                    for p in range(NPAIR)
                ]
                proj_out[kind] = outs
                NTS = S // SQ  # proj t-slices (reuse SQ width)
                for ts in range(NTS):
                    prs = [
                        ps.tile([128, SQ], f32, tag="big", name=f"pr{kind}{ts}_{p}")
                        for p in range(NPAIR)
                    ]
                    for k in range(KT):
                        xt = sb.tile([128, SQ], fr, tag="xt", bufs=3, name=f"xt{kind}")
                        nc.sync.dma_start(
                            xt[:],
                            xT[kind][
                                k * 128 : (k + 1) * 128, ts * SQ : (ts + 1) * SQ
                            ].bitcast(fr),
                        )
                        for p in range(NPAIR):
                            for j in range(SQ // MMN):
                                nc.tensor.matmul(
                                    prs[p][:, j * MMN : (j + 1) * MMN],
                                    (w_sb[p][:, k * 128 : (k + 1) * 128]),
                                    (xt[:, j * MMN : (j + 1) * MMN]),
                                    start=(k == 0),
                                    stop=(k == KT - 1),
                                )
                    for p in range(NPAIR):
                        dst = outs[p][:, ts * SQ : (ts + 1) * SQ]
                        if kind == "q":
                            nc.vector.tensor_copy(dst, prs[p][:])
                        elif kind == "k":
                            nc.vector.tensor_scalar(
                                dst,
                                prs[p][:],
                                1.0 / 8.0,
                                bk_sb[:, p : p + 1],
                                mybir.AluOpType.mult,
                                mybir.AluOpType.add,
                            )
                        else:
                            nc.vector.tensor_scalar_add(
                                dst, prs[p][:], bv_sb[:, p : p + 1]
                            )
            q2T, k2T, v2T = proj_out["q"], proj_out["k"], proj_out["v"]

            # --- phase T: q2 transpose -> q2aug [t, dk|1] per head ---
            q2aug = []
            for h in range(2 * NPAIR):
                p, prow = h // 2, 64 * (h % 2)
                qa = big.tile([128, TT * 65], fa, tag="q2aug", bufs=2 * NPAIR)
                q2aug.append(qa)
                for t in range(TT):
                    nc.vector.tensor_copy(qa[:, t * 65 + 64 : t * 65 + 65], ones128[:])
                for t in range(TT):
                    tr = ps.tile([128, 64], f32, tag="sm", name="tr")
                    nc.tensor.transpose(
                        tr[:],
                        q2T[p][prow : prow + 64, t * 128 : (t + 1) * 128],
                        ident_sb[prow : prow + 64, :],
                    )
                    nc.vector.tensor_copy(qa[:, t * 65 : t * 65 + 64], tr[:])

            # --- phase A: attention per head ---
            headout = [
                big.tile([128, S], f32, tag="big2048", bufs=4, name=f"headout_{p}")
                for p in range(NPAIR)
            ]
            cc_in = [
                dram.tile([128, S], f32, name=f"cc_in_{p}") for p in range(NPAIR)
            ]
            cc_out = [
                dram.tile([GROUP * 128, S], f32, name=f"cc_out_{p}")
                for p in range(NPAIR)
            ]
            for h in range(2 * NPAIR):
                p, prow = h // 2, 64 * (h % 2)
                for sh in range(NSH):
                    expt = []
                    for tb in range(TT):
                        sc = ps.tile([128, SQ], f32, tag="big", name="sc")
                        for j in range(SQ // MMN):
                            nc.tensor.matmul(
                                sc[:, j * MMN : (j + 1) * MMN],
                                (k2T[p][prow : prow + 64, tb * 128 : (tb + 1) * 128]),
                                (
                                    v2T[p][
                                        prow : prow + 64,
                                        sh * SQ + j * MMN : sh * SQ + (j + 1) * MMN,
                                    ]
                                ),
                                start=True,
                                stop=True,
                            )
                        et = sb.tile([128, SQ], fa, tag="expt", bufs=min(TT + 8, 2 * TT), name="et")
                        nc.scalar.activation(
                            et[:], sc[:], mybir.ActivationFunctionType.Exp
                        )
                        expt.append(et)
                    for s2 in range(NSB2):
                        av = ps.tile([65, SB], f32, tag="av", name="av")
                        for tk in range(TT):
                            nc.tensor.matmul(
                                av[:],
                                (q2aug[h][:, tk * 65 : tk * 65 + 65]),
                                (expt[tk][:, s2 * SB : (s2 + 1) * SB]),
                                start=(tk == 0),
                                stop=(tk == TT - 1),
                            )
                        recip = sb.tile([1, SB], f32, tag="recip", bufs=2)
                        nc.vector.tensor_copy(recip[:], av[64:65, :])
                        bc = ps.tile([64, SB], f32, tag="sm", name="bc")
                        nc.tensor.matmul(
                            bc[:], (ones64[:]), (recip[:]), start=True, stop=True
                        )
                        bcs = sb.tile([64, SB], f32, tag="bcs", bufs=2, name="bcs")
                        nc.vector.tensor_copy(bcs[:], bc[:])
                        rcp = sb.tile([64, SB], f32, tag="bcs", bufs=2, name="rcp")
                        nc.vector.reciprocal(rcp[:], bcs[:])
                        dst = headout[p][
                            prow : prow + 64, sh * SQ + s2 * SB : sh * SQ + (s2 + 1) * SB
                        ]
                        nc.vector.tensor_mul(dst, av[0:64, :], rcp[:])
                        nc.vector.tensor_scalar_add(
                            dst, dst, bq_sb[prow : prow + 64, p : p + 1]
                        )
                # pair complete -> gather its head outputs across the group,
                # overlapping the collective with the remaining attention work
                if h % 2 == 1:
                    nc.sync.dma_start(cc_in[p][:], headout[p][:])
                    nc.gpsimd.collective_compute(
                        "AllGather",
                        mybir.AluOpType.bypass,
                        replica_groups=[[0, 1, 2, 3], [4, 5, 6, 7]],
                        ins=[cc_in[p].opt()],
                        outs=[cc_out[p].opt()],
                    )


            # --- phase O: output projection, transposed layout outT[d, s] ---
            # global hdk block k = heads {2k, 2k+1} = cc_out[k % 2] rows
            # [128*(k//2) : +128). Accumulate pair-0 blocks first so the
            # matmuls can start as soon as the first AllGather lands.
            korder = [k for k in range(KO) if k % 2 == 0] + [
                k for k in range(KO) if k % 2 == 1
            ]
            OSW = min(512, S)
            for sblk in range(S // OSW):
                pos = [
                    ps.tile([128, OSW], f32, tag="av", name=f"po{d}") for d in range(2)
                ]
                for ki, k in enumerate(korder):
                    ch = sb.tile([128, OSW], fr, tag="ch", bufs=3, name="ch")
                    nc.sync.dma_start(
                        ch[:],
                        cc_out[k % 2][
                            128 * (k // 2) : 128 * (k // 2) + 128,
                            sblk * OSW : (sblk + 1) * OSW,
                        ].bitcast(fr),
                    )
                    for dblk in range(2):
                        nc.tensor.matmul(
                            pos[dblk][:],
                            woT_sb[:, k * DSL + 128 * dblk : k * DSL + 128 * (dblk + 1)],
                            ch[:],
                            start=(ki == 0),
                            stop=(ki == KO - 1),
                        )
                for dblk in range(2):
                    ob = sb.tile([128, OSW], f32, tag="ob", bufs=3, name="ob")
                    nc.vector.tensor_scalar_add(
                        ob[:], pos[dblk][:], boT_sb[:, dblk : dblk + 1]
                    )
                    nc.sync.dma_start(
                        out_ap[
                            128 * dblk : 128 * (dblk + 1), sblk * OSW : (sblk + 1) * OSW
                        ],
                        ob[:],
                    )

    if split_waits:
        _split_multi_waits(nc)
    return nc


def make_in_maps(v, k, q, Wq, bqv, Wk, bkv, Wv, bvv, Wo, bov, S, DM, H, DK):
    """Per-core input dicts from full inputs (all host-side prep is slicing /
    transpose / trivial broadcast)."""
    HDK = H * DK
    DSL = HDK // GROUP
    xT = {}
    for b in range(2):
        xT[("q", b)] = np.ascontiguousarray(q[b].T)  # [DM, S]
        xT[("k", b)] = np.ascontiguousarray(k[b].T)
        xT[("v", b)] = np.ascontiguousarray(v[b].T)
    WoT = np.ascontiguousarray(Wo.T)  # [HDK, HDK_out]
    ident = np.vstack([np.eye(64, dtype=np.float32)] * 2)
    in_maps = []
    for c in range(N_CORES):
        b = c // GROUP
        h0 = 4 * (c % GROUP)
        m = {
            "xqT": xT[("q", b)],
            "xkT": xT[("k", b)],
            "xvT": xT[("v", b)],
            "ident": ident,
        }
        for kind, Wt, bt in (("q", Wq, bqv), ("k", Wk, bkv), ("v", Wv, bvv)):
            wp = np.empty((2, DM, 128), np.float32)
            bp = np.empty((2, 128, 1), np.float32)
            for p in range(2):
                ha, hb = h0 + 2 * p, h0 + 2 * p + 1
                wp[p, :, :64] = Wt[ha].T
                wp[p, :, 64:] = Wt[hb].T
                bp[p, :64, 0] = bt[ha]
                bp[p, 64:, 0] = bt[hb]
            m[f"w{kind}"] = wp
            if kind == "q":
                m["bq"] = bp
            elif kind == "k":
                m["bk8"] = bp / 8.0
            else:
                m["bv"] = bp
        d0 = DSL * (c % GROUP)
        m["woT"] = np.ascontiguousarray(WoT[:, d0 : d0 + DSL])
        m["boT"] = np.ascontiguousarray(bov[d0 : d0 + DSL].reshape(2, 128).T)
        in_maps.append(m)
    return in_maps


def kernel(v, k, q, Wq, bq, Wk, bk, Wv, bv, Wo, bo, _trace=False):
    """Full inputs in, full output out. Runs the SPMD Bass kernel on 8 cores."""
    global last_results
    from concourse.bass_utils import run_bass_kernel_spmd

    v, k, q = (np.asarray(a, np.float32) for a in (v, k, q))
    B, S, DM = q.shape
    H, DK = Wq.shape[0], Wq.shape[1]
    HDK = H * DK
    DSL = HDK // GROUP

    nc = build_program(S=S, DM=DM, H=H, DK=DK)
    in_maps = make_in_maps(
        np.asarray(v, np.float32),
        np.asarray(k, np.float32),
        np.asarray(q, np.float32),
        *(np.asarray(a, np.float32) for a in (Wq, bq, Wk, bk, Wv, bv, Wo, bo)),
        S=S,
        DM=DM,
        H=H,
        DK=DK,
    )
    res = run_bass_kernel_spmd(nc, in_maps, list(range(N_CORES)), trace=_trace)
    last_results = res
    out = np.empty((B, S, HDK), np.float32)
    for c in range(N_CORES):
        b = c // GROUP
        d0 = DSL * (c % GROUP)
        out[b, :, d0 : d0 + DSL] = res.results[c]["out"].T
    return out



# revision 19
# speedup vs baseline: 1.5332x; 1.5332x over previous
"""Trainium2 Bass kernel for nn_MultiHeadAttention_56118042690041.

8-core sharding: batch x heads tensor-parallel.
  core c (0..7): batch b = c//4, heads 4*(c%4) .. 4*(c%4)+4 (as 2 packed pairs).

v2 pipeline (all-bf16 dataflow, fp32 PSUM accumulation):
  - x cast to bf16 on host (halves HBM traffic); all weights bf16.
  - v/k projections: W-stationary matmuls, bias (+1/8 scale for k) on DVE,
    outputs kept bf16 in SBUF as [128(dk pair), S].
  - q projection emitted TRANSPOSED (x-stationary, W moving) directly into the
    [t, dk] layout AV needs -- no PE transpose phase. The per-head ones column
    (softmax denominator trick) comes from memsetting the qa tiles to 1.0.
  - attention: slot pipeline over (head, t-block, s-half). Per slot the PE runs
    2 score matmuls (K=64) into a double-buffered PSUM tile and 2 AV matmuls of
    the PREVIOUS head (K=128, tk-outer so exp tiles free early), while ACT runs
    exp back-to-back (the true bottleneck: 4*S*S/128 elem-cycles ~ 109us).
  - softmax normalize: reciprocal of the denominator row, PE K=1 f32r matmul
    broadcasts it into the unused partitions of the same av PSUM tile, DVE
    multiply + bq add -> headout bf16.
  - per-pair bf16 AllGather of head outputs across the 4-core batch group,
    pair 0 overlapped under the second half of attention.
  - output projection: korder pair0-first so matmuls start before the second
    AllGather lands; disjoint 256-wide d-slice per core (column-sharded Wo).
"""

import contextlib
import ctypes
import os
import sys
import types

import ml_dtypes
import numpy as np

if "/opt/trn_rl_repo" not in sys.path:
    sys.path.insert(0, "/opt/trn_rl_repo")

# ---------------------------------------------------------------- shims ----


def _install_antenv_shim():
    """Provide antenv.axon_hooks (NTFF profile hook) if the image lacks it."""
    try:
        import antenv.axon_hooks  # noqa: F401

        return
    except ImportError:
        pass

    def _hook_factory():
        so_path = "/opt/axon/libaxon_pjrt.so"
        try:
            lib = ctypes.CDLL(so_path)
        except OSError:
            return None
        if not hasattr(lib, "axon_start_nrt_profile"):
            return None
        lib.axon_start_nrt_profile.argtypes = [
            ctypes.POINTER(ctypes.c_int64),
            ctypes.c_size_t,
        ]
        lib.axon_start_nrt_profile.restype = ctypes.c_int64
        lib.axon_stop_nrt_profile.argtypes = [ctypes.c_char_p]
        lib.axon_stop_nrt_profile.restype = ctypes.c_int64

        @contextlib.contextmanager
        def _hook(output_dir, device_ids):
            import jax

            jax.devices()
            if device_ids:
                ids = (ctypes.c_int64 * len(device_ids))(*device_ids)
                rc = lib.axon_start_nrt_profile(ids, len(device_ids))
            else:
                rc = lib.axon_start_nrt_profile(None, 0)
            if rc != 0:
                raise RuntimeError(f"axon_start_nrt_profile rc={rc}")
            try:
                yield
            finally:
                n = lib.axon_stop_nrt_profile(str(output_dir).encode())
                print(f"ntff profile: {n} file(s) -> {output_dir}", file=sys.stderr)

        return _hook

    hook = _hook_factory()
    mod = types.ModuleType("antenv.axon_hooks")
    mod.get_axon_ntff_profile_hook = lambda: hook
    mod.set_axon_ntff_profile_hook = lambda h: None
    sys.modules["antenv.axon_hooks"] = mod


def _install_tile_drain_patch():
    """This walrus build rejects >1 sync wait on the Tile tail Drain; split the
    waits across chained single-wait drains."""
    import concourse.tile as tile

    if getattr(tile.TileContext, "_drain_patch_installed", False):
        return

    def _drain_and_barrier(self, tick_clock, wait_clock):
        nc = self.nc
        drain_inst = nc.sync.drain()
        wait_clock.add_sem_waits(
            drain_inst.ins, tile.ScopedClock({None: tick_clock.global_clock})
        )
        si = drain_inst.ins.sync_info
        waits = list(si.on_wait) if si is not None and si.on_wait else []
        if len(waits) > 1:
            si.on_wait = waits[:1]
            assert self.sems is not None
            by_num = {h.num: h for h in self.sems.allocated().values()}
            for w in waits[1:]:
                d2 = nc.sync.drain()
                h = by_num.get(w.id)
                assert h is not None, f"no sem handle for wait {w.ant_name}"
                d2.wait_op(h, w.wait_value, "sem-ge", check=False)
        nc.all_engine_barrier()
        assert self.sems is not None
        popped = nc._tile_sem_poison_stack.pop()
        assert popped is self._sem_poison
        nc.clear_and_free_semaphores(list(self.sems.allocated().values()))
        nc.all_engine_barrier()

    tile.TileContext._drain_and_barrier = _drain_and_barrier
    tile.TileContext._drain_patch_installed = True


_install_antenv_shim()


def _split_multi_waits(nc, max_waits=1):
    """This walrus build rejects instructions carrying more than ~1 sync wait.
    Move excess waits onto same-engine NOPs inserted immediately before the
    instruction (sequencer waits execute in stream order, so this is
    semantics-preserving)."""
    import bass_rust
    import concourse.mybir as mybir

    n = 0
    for bb in nc.m.functions[0].blocks:
        insts = bb.instructions
        out = []
        for inst in insts:
            si = inst.sync_info
            waits = list(si.on_wait) if si is not None and si.on_wait else []
            if len(waits) > max_waits:
                keep = waits[-max_waits:]
                for w in waits[:-max_waits]:
                    nop = mybir.InstNoOp(name=f"waitnop_{n}", ins=[], outs=[])
                    n += 1
                    nop.engine = inst.engine
                    nop.sync_info = bass_rust.SyncInfo(on_wait=[w], on_update=[])
                    out.append(nop)
                si.on_wait = keep
            out.append(inst)
        if len(out) != len(insts):
            insts[:] = out
    return n


# ------------------------------------------------------------- program -----

N_CORES = 8
GROUP = 4  # cores per batch group

last_results = None  # BassKernelResults of the most recent run (for test.py)


def build_program(S=2048, DM=1024, H=16, DK=64, split_waits=True):
    """Emit the SPMD Bass/Tile program. Returns nc."""
    import concourse.bass as bass
    import concourse.mybir as mybir
    import concourse.tile as tile

    _install_tile_drain_patch()

    f32 = mybir.dt.float32
    f32r = mybir.dt.float32r
    bf16 = mybir.dt.bfloat16
    NPAIR = 2  # head pairs per core (4 heads)
    NH = 2 * NPAIR  # heads per core
    KT = DM // 128  # contraction tiles for projections
    TT = S // 128  # t-blocks
    NS2 = S // 512  # AV s-blocks
    HDK = H * DK
    KO = HDK // 128  # outproj contraction tiles
    DSL = HDK // GROUP  # out d-slice per core (256)

    nc = bass.Bass(
        trn_type="TRN2", target_bir_lowering=False, debug=False, num_devices=N_CORES
    )

    def din(name, shape, dt=bf16):
        return nc.dram_tensor(name, shape, dt, kind="ExternalInput").ap()

    xT = {p: din(f"x{p}T", [DM, S]) for p in ("q", "k", "v")}  # x[b].T, bf16
    W = {p: din(f"w{p}", [NPAIR, DM, 128]) for p in ("k", "v")}  # pair-packed W.T
    wq4 = din("wq4", [DM, NH * DK])  # Wq.T all 4 local heads side by side
    bq = din("bq", [NPAIR, 128, 1], f32)
    bk8 = din("bk8", [NPAIR, 128, 1], f32)  # bk / sqrt(dk)
    bv = din("bv", [NPAIR, 128, 1], f32)
    woT = din("woT", [HDK, DSL])  # Wo.T columns for this core's d-slice
    boT = din("boT", [128, 2], f32)  # bo d-slice as [128, 2]

    out_ap = nc.dram_tensor("out", [DSL, S], f32, kind="ExternalOutput").ap()

    with tile.TileContext(nc) as tc:
        with contextlib.ExitStack() as ctx:
            sb = ctx.enter_context(tc.tile_pool(name="sb", bufs=2))
            big = ctx.enter_context(tc.tile_pool(name="big", bufs=8))
            ps = ctx.enter_context(tc.tile_pool(name="ps", bufs=2, space="PSUM"))
            dram = ctx.enter_context(tc.tile_pool(name="dram", bufs=1, space="DRAM"))

            # --- constants / weights ---
            ones64 = sb.tile([1, 64], bf16, tag="ones", bufs=1)
            nc.gpsimd.memset(ones64[:], 1.0)
            bq_sb = sb.tile([128, NPAIR], f32, tag="bq", bufs=1)
            bk_sb = sb.tile([128, NPAIR], f32, tag="bk", bufs=1)
            bv_sb = sb.tile([128, NPAIR], f32, tag="bv", bufs=1)
            for p in range(NPAIR):
                nc.sync.dma_start(bq_sb[:, p : p + 1], bq[p])
                nc.sync.dma_start(bk_sb[:, p : p + 1], bk8[p])
                nc.sync.dma_start(bv_sb[:, p : p + 1], bv[p])
            boT_sb = sb.tile([128, 2], f32, tag="bo", bufs=1)
            nc.sync.dma_start(boT_sb[:], boT[:])

            w_sb = {}
            for kind in ("v", "k"):
                w_sb[kind] = [
                    sb.tile([128, KT * 128], bf16, tag="w", bufs=4, name=f"w_{kind}{p}")
                    for p in range(NPAIR)
                ]
                for p in range(NPAIR):
                    for kk in range(KT):
                        nc.sync.dma_start(
                            w_sb[kind][p][:, kk * 128 : (kk + 1) * 128],
                            W[kind][p, kk * 128 : (kk + 1) * 128, :],
                        )
            wq_sb = sb.tile([128, KT * 256], bf16, tag="wq", bufs=1)
            for kk in range(KT):
                nc.sync.dma_start(
                    wq_sb[:, kk * 256 : (kk + 1) * 256],
                    wq4[kk * 128 : (kk + 1) * 128, :],
                )
            woT_sb = sb.tile([128, KO * DSL], bf16, tag="wo", bufs=1)
            for kk in range(KO):
                nc.sync.dma_start(
                    woT_sb[:, kk * DSL : (kk + 1) * DSL],
                    woT[kk * 128 : (kk + 1) * 128, :],
                )

            # qa tiles: per pair [128, TT*130] layout per t-block:
            #   [headA 64 | onesA 1 | headB 64 | onesB 1]
            # memset to 1.0 so the ones columns are ready; q-proj fills heads.
            qa = [
                big.tile([128, TT * 130], bf16, tag="qa", bufs=NPAIR, name=f"qa{p}")
                for p in range(NPAIR)
            ]
            for p in range(NPAIR):
                nc.gpsimd.memset(qa[p][:], 1.0)

            # --- phase P: v/k projections -> pair tiles [128, S] bf16 ---
            proj_out = {}
            for kind in ("v", "k"):
                outs = [
                    big.tile([128, S], bf16, tag="vk", bufs=4, name=f"{kind}2T_{p}")
                    for p in range(NPAIR)
                ]
                proj_out[kind] = outs
                for ts in range(S // 512):
                    prs = [
                        ps.tile(
                            [128, 512], f32, tag="ps512", bufs=4, name=f"pr{kind}{p}"
                        )
                        for p in range(NPAIR)
                    ]
                    for kk in range(KT):
                        xt = sb.tile([128, 512], bf16, tag="xt", bufs=6, name="xt")
                        nc.sync.dma_start(
                            xt[:],
                            xT[kind][
                                kk * 128 : (kk + 1) * 128, ts * 512 : (ts + 1) * 512
                            ],
                        )
                        for p in range(NPAIR):
                            nc.tensor.matmul(
                                prs[p][:],
                                w_sb[kind][p][:, kk * 128 : (kk + 1) * 128],
                                xt[:],
                                start=(kk == 0),
                                stop=(kk == KT - 1),
                            )
                    for p in range(NPAIR):
                        dst = outs[p][:, ts * 512 : (ts + 1) * 512]
                        if kind == "k":
                            nc.vector.tensor_scalar(
                                dst,
                                prs[p][:],
                                1.0 / 8.0,
                                bk_sb[:, p : p + 1],
                                mybir.AluOpType.mult,
                                mybir.AluOpType.add,
                            )
                        else:
                            nc.vector.tensor_scalar_add(
                                dst, prs[p][:], bv_sb[:, p : p + 1]
                            )
            v2T, k2T = proj_out["v"], proj_out["k"]

            # --- phase A: slot-pipelined attention + transposed q-proj ---
            headout = [
                big.tile([128, S], bf16, tag="ho", bufs=NPAIR, name=f"headout_{p}")
                for p in range(NPAIR)
            ]
            cc_in = [
                dram.tile([128, S], bf16, name=f"cc_in_{p}") for p in range(NPAIR)
            ]
            cc_out = [
                dram.tile([GROUP * 128, S], bf16, name=f"cc_out_{p}")
                for p in range(NPAIR)
            ]

            ET_BUFS = 20
            pend = []  # deferred norm pieces, flushed at the next slot start
            et_tiles = {}  # (h, tb) -> tile (fresh ring allocation per head)

            def et_get(h, tb):
                key = (h, tb)
                if key not in et_tiles:
                    et_tiles[key] = big.tile(
                        [128, S], bf16, tag="et", bufs=ET_BUFS, name=f"et{h}_{tb}"
                    )
                return et_tiles[key]

            def emit_scores(h, tb, half, sc):
                p, prow = h // 2, 64 * (h % 2)
                for j in range(2):
                    nc.tensor.matmul(
                        sc[:, j * 512 : (j + 1) * 512],
                        k2T[p][prow : prow + 64, tb * 128 : (tb + 1) * 128],
                        v2T[p][
                            prow : prow + 64,
                            half * 1024 + j * 512 : half * 1024 + (j + 1) * 512,
                        ],
                        start=True,
                        stop=True,
                    )

            def emit_qproj(tb, half, qp, xq_tiles):
                # accumulate Wq over 4 of 8 ktiles into qp[:, :256]
                g, sub = tb // 4, tb % 4
                for kk in range(half * 4, half * 4 + 4):
                    nc.tensor.matmul(
                        qp[:, 0:256],
                        xq_tiles[(g, kk)][:, sub * 128 : (sub + 1) * 128],
                        wq_sb[:, kk * 256 : (kk + 1) * 256],
                        start=(kk == 0),
                        stop=(kk == KT - 1),
                    )

            def emit_qcopy(tb, qp):
                for hh in range(NH):
                    p = hh // 2
                    dst = qa[p][
                        :, tb * 130 + 65 * (hh % 2) : tb * 130 + 65 * (hh % 2) + 64
                    ]
                    nc.vector.tensor_copy(dst, qp[:, hh * 64 : (hh + 1) * 64])

            def emit_av_step(hprev, step, av_tiles):
                # tk-outer ordering: matmul m = tk*NS2 + s2, 2 per slot
                pp = hprev // 2
                qoff = 65 * (hprev % 2)
                for m in (2 * step, 2 * step + 1):
                    tk, s2 = m // NS2, m % NS2
                    nc.tensor.matmul(
                        av_tiles[s2][0:65, :],
                        qa[pp][:, tk * 130 + qoff : tk * 130 + qoff + 65],
                        et_get(hprev, tk)[:, s2 * 512 : (s2 + 1) * 512],
                        start=(tk == 0),
                        stop=(tk == TT - 1),
                    )
                    if tk == TT - 1:
                        emit_norm_a(hprev, s2, av_tiles[s2])

            def emit_norm_a(h, s2, av):
                # reciprocal of denominator row (DVE), then defer bc + mul
                rcp1 = sb.tile([1, 512], bf16, tag="rcp", bufs=4, name=f"rcp{s2 % 4}")
                with nc.allow_low_precision(reason="1/den in bf16 for PE broadcast"):
                    nc.vector.reciprocal(rcp1[:], av[64:65, :])
                pend.append((h, s2, av, rcp1))

            def flush_pend():
                while pend:
                    h, s2, av, rcp1 = pend.pop(0)
                    p, prow = h // 2, 64 * (h % 2)
                    # broadcast 1/den into rows 64:128 of the same PSUM tile
                    nc.tensor.matmul(
                        av[64:128, :],
                        ones64[:],
                        rcp1[:],
                        start=True,
                        stop=True,
                    )
                    # DVE tensor_tensor cannot take two PSUM operands; stage
                    # the broadcast through SBUF.
                    bcs = sb.tile([64, 512], f32, tag="bcs", bufs=4, name=f"bcs{s2 % 4}")
                    nc.vector.tensor_copy(bcs[:], av[64:128, :])
                    dst = headout[p][prow : prow + 64, s2 * 512 : (s2 + 1) * 512]
                    nc.vector.tensor_mul(dst, av[0:64, :], bcs[:])
                    nc.vector.tensor_scalar_add(dst, dst, bq_sb[prow : prow + 64, p : p + 1])

            def emit_ag(p):
                nc.sync.dma_start(cc_in[p][:], headout[p][:])
                nc.gpsimd.collective_compute(
                    "AllGather",
                    mybir.AluOpType.bypass,
                    replica_groups=[[0, 1, 2, 3], [4, 5, 6, 7]],
                    ins=[cc_in[p].opt()],
                    outs=[cc_out[p].opt()],
                )

            xq_tiles = {}
            av_cur = None
            for h in range(NH):
                # av tiles for the PREVIOUS head's AV accumulation
                if h >= 1:
                    av_cur = [
                        ps.tile([128, 512], f32, tag="ps512", bufs=4, name=f"av{s2}")
                        for s2 in range(NS2)
                    ]
                for tb in range(TT):
                    if h == 0:
                        # prefetch xq tiles for stile group g = tb//4
                        if tb % 4 == 0:
                            g = tb // 4
                            for kk in range(KT):
                                t = sb.tile(
                                    [128, 512], bf16, tag="xq", bufs=10,
                                    name=f"xq{(g * KT + kk) % 10}",
                                )
                                nc.sync.dma_start(
                                    t[:],
                                    xT["q"][
                                        kk * 128 : (kk + 1) * 128,
                                        g * 512 : (g + 1) * 512,
                                    ],
                                )
                                xq_tiles[(g, kk)] = t
                        qp = ps.tile([128, 512], f32, tag="ps512", bufs=4, name="qp")
                    et = et_get(h, tb)
                    for half in range(2):
                        flush_pend()  # previous slot's norms (bc + mul + bias)
                        # PE fillers (no dependence on the sc ring)
                        if h == 0:
                            emit_qproj(tb, half, qp, xq_tiles)
                        else:
                            emit_av_step(h - 1, tb * 2 + half, av_cur)
                        sc = ps.tile([128, 1024], f32, tag="sc", bufs=2, name="sc")
                        emit_scores(h, tb, half, sc)
                        nc.scalar.activation(
                            et[:, half * 1024 : (half + 1) * 1024],
                            sc[:],
                            mybir.ActivationFunctionType.Exp,
                        )
                    if h == 0:
                        emit_qcopy(tb, qp)
                    # AG0 fires early in head 3's slots: by then AV(1) norms
                    # (flushed at tb==0) completed headout[0] (= pair 0).
                    if h == 3 and tb == 1:
                        emit_ag(0)

            # tail: AV + norms of the last head (PE-only, ACT idle)
            av_cur = [
                ps.tile([128, 512], f32, tag="ps512", bufs=4, name=f"av{s2}")
                for s2 in range(NS2)
            ]
            for step in range(2 * TT):
                flush_pend()
                emit_av_step(NH - 1, step, av_cur)
            flush_pend()
            emit_ag(1)

            # --- phase O: output projection, transposed layout outT[d, s] ---
            korder = [k for k in range(KO) if k % 2 == 0] + [
                k for k in range(KO) if k % 2 == 1
            ]
            for sblk in range(NS2):
                pos = [
                    ps.tile([128, 512], f32, tag="ps512", bufs=4, name=f"po{d}")
                    for d in range(2)
                ]
                for ki, k in enumerate(korder):
                    ch = sb.tile([128, 512], bf16, tag="ch", bufs=6, name="ch")
                    nc.sync.dma_start(
                        ch[:],
                        cc_out[k % 2][
                            128 * (k // 2) : 128 * (k // 2) + 128,
                            sblk * 512 : (sblk + 1) * 512,
                        ],
                    )
                    for dblk in range(2):
                        nc.tensor.matmul(
                            pos[dblk][:],
                            woT_sb[:, k * DSL + 128 * dblk : k * DSL + 128 * (dblk + 1)],
                            ch[:],
                            start=(ki == 0),
                            stop=(ki == KO - 1),
                        )
                for dblk in range(2):
                    ob = sb.tile([128, 512], f32, tag="ob", bufs=3, name="ob")
                    nc.vector.tensor_scalar_add(
                        ob[:], pos[dblk][:], boT_sb[:, dblk : dblk + 1]
                    )
                    nc.sync.dma_start(
                        out_ap[
                            128 * dblk : 128 * (dblk + 1), sblk * 512 : (sblk + 1) * 512
                        ],
                        ob[:],
                    )

    if split_waits:
        _split_multi_waits(nc)
    return nc


def make_in_maps(v, k, q, Wq, bqv, Wk, bkv, Wv, bvv, Wo, bov, S, DM, H, DK):
    """Per-core input dicts from full inputs (host prep: slice/transpose/cast)."""
    bf16 = ml_dtypes.bfloat16
    HDK = H * DK
    DSL = HDK // GROUP
    xT = {}
    for b in range(2):
        xT[("q", b)] = np.ascontiguousarray(q[b].T).astype(bf16)
        xT[("k", b)] = np.ascontiguousarray(k[b].T).astype(bf16)
        xT[("v", b)] = np.ascontiguousarray(v[b].T).astype(bf16)
    WoT = np.ascontiguousarray(Wo.T)  # [HDK, HDK_out]
    in_maps = []
    for c in range(N_CORES):
        b = c // GROUP
        h0 = 4 * (c % GROUP)
        m = {
            "xqT": xT[("q", b)],
            "xkT": xT[("k", b)],
            "xvT": xT[("v", b)],
        }
        for kind, Wt, bt in (("k", Wk, bkv), ("v", Wv, bvv)):
            wp = np.empty((2, DM, 128), np.float32)
            bp = np.empty((2, 128, 1), np.float32)
            for p in range(2):
                ha, hb = h0 + 2 * p, h0 + 2 * p + 1
                wp[p, :, :64] = Wt[ha].T
                wp[p, :, 64:] = Wt[hb].T
                bp[p, :64, 0] = bt[ha]
                bp[p, 64:, 0] = bt[hb]
            m[f"w{kind}"] = wp.astype(bf16)
            if kind == "k":
                m["bk8"] = (bp / 8.0).astype(np.float32)
            else:
                m["bv"] = bp.astype(np.float32)
        # q: all 4 local heads side by side [DM, 256]
        wq4 = np.empty((DM, 256), np.float32)
        bqp = np.empty((2, 128, 1), np.float32)
        for hh in range(4):
            wq4[:, hh * 64 : (hh + 1) * 64] = Wq[h0 + hh].T
        for p in range(2):
            bqp[p, :64, 0] = bqv[h0 + 2 * p]
            bqp[p, 64:, 0] = bqv[h0 + 2 * p + 1]
        m["wq4"] = wq4.astype(bf16)
        m["bq"] = bqp.astype(np.float32)
        d0 = DSL * (c % GROUP)
        m["woT"] = np.ascontiguousarray(WoT[:, d0 : d0 + DSL]).astype(bf16)
        m["boT"] = np.ascontiguousarray(bov[d0 : d0 + DSL].reshape(2, 128).T).astype(
            np.float32
        )
        in_maps.append(m)
    return in_maps


def kernel(v, k, q, Wq, bq, Wk, bk, Wv, bv, Wo, bo, _trace=False):
    """Full inputs in, full output out. Runs the SPMD Bass kernel on 8 cores."""
    global last_results
    from concourse.bass_utils import run_bass_kernel_spmd

    v, k, q = (np.asarray(a, np.float32) for a in (v, k, q))
    B, S, DM = q.shape
    H, DK = Wq.shape[0], Wq.shape[1]
    HDK = H * DK
    DSL = HDK // GROUP

    nc = build_program(S=S, DM=DM, H=H, DK=DK)
    in_maps = make_in_maps(
        v,
        k,
        q,
        *(np.asarray(a, np.float32) for a in (Wq, bq, Wk, bk, Wv, bv, Wo, bo)),
        S=S,
        DM=DM,
        H=H,
        DK=DK,
    )
    res = run_bass_kernel_spmd(nc, in_maps, list(range(N_CORES)), trace=_trace)
    last_results = res
    out = np.empty((B, S, HDK), np.float32)
    for c in range(N_CORES):
        b = c // GROUP
        d0 = DSL * (c % GROUP)
        out[b, :, d0 : d0 + DSL] = res.results[c]["out"].T
    return out
